# revision 19
# baseline (speedup 1.0000x reference)
"""Cross-attention Trainium2 kernel (8 NeuronCores, data-parallel over batch).

Reference computation per batch element b:
    x_flat = x[b].reshape(C, N).T                 # [N, C], N = H*W = 4096
    q = x_flat @ Wq ; k = ctx @ Wk ; v = ctx @ Wv  # heads=8, dim_head=64
    attn = softmax(q k^T / 8) ; o = attn v
    out = (o @ Wo + bo).T.reshape(C, H, W) + x[b]

Device layout (everything transposed so the HW-native [C, N] layout of x is
used directly; S^T = k q^T kept with m (context tokens) on partitions so the
softmax denominator comes free from an extra ones-row in v'):

  qT  [inner, 512]  = Wq^T x[:, nc]   streamed per n-chunk
  kT  [inner, M]    = Wk^T ctx^T      persistent
  v'  [M, 8*65]     = (ctx Wv | 1)    per-head 65-wide blocks: 64 v-cols + ones
  S^T [m 128, n]    = kT_h^T qT_h     fp32r matmuls, K=64
  P^T = exp(S^T * 0.125)              ScalarE, f32r out
  O'  [65, n]       = v'^T P^T        matmuls, K=128, accumulated over m;
                                      row 64 = softmax denominators
  out = Wo^T (O'/sums) + bo + x       bf16 matmuls + DVE epilogue

Schedule: software-pipelined so the ScalarE exp stream (the bottleneck
engine: ~266us of irreducible exp work) never stalls.  Per n-chunk, per
head: 4 QK groups of [128,1024] S-PSUM (3 rotating 2-bank tiles), exp per
group, then 8 AV matmuls into a [65,512] accumulator (2 rotating banks).
Q-projection i-chunks for chunk nci+1 are emitted during heads 0-3, the
out-projection of chunk nci-1 during heads 4-7 (reusing the qproj x tile
for the residual add), and the softmax-denominator epilogue of chunk nci-1
(reciprocal + broadcast + normalize) is spread over heads 1-4 so no DVE
instruction ever heads the queue while waiting on a DMA.
"""

import sys

for _p in ("/opt/trn_rl_repo", "/opt/pypackages"):
    if _p not in sys.path:
        sys.path.append(_p)

import numpy as np

import concourse.bass as bass
import concourse.tile as tile
from concourse import bacc, mybir
from concourse.bass_utils import run_bass_kernel_spmd

B, C, HH, WW = 8, 512, 64, 64
N = HH * WW            # 4096 query tokens
M = 1024               # context tokens
CTX = 768              # context channels
HEADS, DH = 8, 64
INNER = HEADS * DH     # 512
SCALE = DH ** -0.5     # 0.125

F32 = mybir.dt.float32
F32R = mybir.dt.float32r
BF16 = mybir.dt.bfloat16
FP8 = mybir.dt.float8e4
DR = mybir.MatmulPerfMode.DoubleRow
EXP_BIAS = -3.5        # exp(s-2): keeps P in fp8e4 range (TRN max 240);
                       # cancels exactly in O/sum normalization
AF = mybir.ActivationFunctionType
ALU = mybir.AluOpType

NCI = N // 512         # 8 n-chunks of 512
MCI = M // 128         # 8 m-chunks of 128
CCH = C // 128         # 4
XCH = CTX // 128       # 6
NG = 4                 # QK/exp groups per head (2 m-chunks each)

_PROG = None
_DEBUG = False


def _build(reps=1):
    nc = bacc.Bacc("TRN2", target_bir_lowering=False, debug=False, num_devices=8)

    x_d = nc.dram_tensor("x", [C, N], F32R, kind="ExternalInput")
    ctxt_d = nc.dram_tensor("ctxT", [CTX, M], F32R, kind="ExternalInput")
    wq_d = nc.dram_tensor("wq", [C, INNER], F32R, kind="ExternalInput")
    wk_d = nc.dram_tensor("wk", [CTX, INNER], F32R, kind="ExternalInput")
    wv_d = nc.dram_tensor("wv", [CTX, INNER], F32R, kind="ExternalInput")
    wo_d = nc.dram_tensor("wo", [INNER, C], F32, kind="ExternalInput")
    bo_d = nc.dram_tensor("bo", [C], F32, kind="ExternalInput")
    out_d = nc.dram_tensor("out", [C, N], F32, kind="ExternalOutput")
    # DRAM bounce for the reciprocal row-broadcast (SBUF APs can't have
    # zero partition step; DRAM APs can)
    rscr_d = nc.dram_tensor("rscr", [64, 512], BF16)
    dbg = {}
    if _DEBUG:
        for nm, shp, dt in [("d_qt", [64, 1024], FP8),
                            ("d_kt8", [64, 2048], FP8),
                            ("d_pt", [128, 1024], FP8),
                            ("d_sall", [1, 4096], F32),
                            ("d_onc", [128, 512], BF16),
                            ("d_rr", [128, 2048], BF16),
                            ("d_v2", [128, 1040], FP8),
                            ("d_x8", [128, 2048], FP8),
                            ("d_qf", [128, 512], FP8),
                            ("d_wq8", [128, 2048], FP8)]:
            dbg[nm] = nc.dram_tensor(nm, shp, dt, kind="ExternalOutput")

    with tile.TileContext(nc) as tc:
        with (
            tc.tile_pool(name="pers", bufs=1) as pers,
            tc.tile_pool(name="xs", bufs=3) as xs,
            tc.tile_pool(name="qp", bufs=8) as qp,
            tc.tile_pool(name="pp", bufs=7) as pp,
            tc.tile_pool(name="misc", bufs=2) as misc,
            tc.tile_pool(name="rrp", bufs=1) as rrp,
            tc.tile_pool(name="sump", bufs=2) as sump,
            tc.tile_pool(name="ost", bufs=2) as ostp,
            tc.tile_pool(name="onc", bufs=2) as oncp,
            tc.tile_pool(name="outp", bufs=2) as outp,
            tc.tile_pool(name="psS", bufs=3, space="PSUM") as psS,
            tc.tile_pool(name="psA", bufs=2, space="PSUM") as psA,
        ):
            for _rep in range(reps):
                # ---------------- persistent tiles + weight loads ----------------
                wo_bf = [pers.tile([128, C], BF16, tag=f"wob{k}", name=f"wob{k}")
                         for k in range(CCH)]
                bo_sb = pers.tile([128, CCH], F32, tag="bo")
                ones_sb = pers.tile([128, 8], F32, tag="ones")
                nc.vector.memset(ones_sb[:], 1.0)
                ebias = pers.tile([128, 1], F32, tag="ebias")
                nc.vector.memset(ebias[:], EXP_BIAS)
                # kt8[i]: [64, 2, 1024] fp8 — partition p holds dh pair
                # (2p, 2p+1) of heads 2i (parts 0-31) / 2i+1 (parts 32-63)
                kt8 = [pers.tile([64, 2048], FP8, tag=f"kt8{i}", name=f"kt8{i}")
                       for i in range(CCH)]
                # v2[g]: m-chunk pair (2g, 2g+1); per member 8 heads x 66
                # (64 v-cols + ones + pad so the member stride is 16-aligned)
                v2_sb = [pers.tile([128, 2 * 8 * 66], FP8, tag=f"v2{g}",
                                   name=f"v2{g}") for g in range(NG)]

                # p-major 3D views: DMA pairs flat iteration order, so the
                # source must iterate (p, a, i) to match the [p, (a i)] tiles
                wkr = wk_d.rearrange("(a p) i -> p a i", p=128)
                wvr = wv_d.rearrange("(a p) i -> p a i", p=128)
                wqr = wq_d.rearrange("(a p) i -> p a i", p=128)
                wor = wo_d.rearrange("(a p) i -> a p i", p=128)
                ctxr = ctxt_d.rearrange("(a p) m -> p a m", p=128)
                xpr = x_d.rearrange("(a p) n -> p a n", p=128)

                def qproj_load(nci):
                    """one 3D-AP DMA fetches all 4 c-chunks of x[:, nsl],
                    plus a DVE cast to the fp8 copy used by the projection."""
                    nsl = slice(nci * 512, (nci + 1) * 512)
                    t = xs.tile([128, 2048], F32R, tag="x", name="xt")
                    nc.sync.dma_start(out=t, in_=xpr[:, :, nsl])
                    t8 = xs.tile([128, 2048], FP8, tag="x8", name="xt8")
                    nc.vector.tensor_copy(t8[:], t[:])
                    return t, t8

                qts0_dbg = [None]

                def qproj_i(i, x8, qts):
                    """one i-chunk of qT = Wq^T x[:, nsl] (fp8 DoubleRow),
                    emitted as the dh-paired fp8 layout [64, 2, 512]."""
                    acc = psA.tile([128, 512], F32, tag="acc", name="qacc")
                    x8v = x8.rearrange("p (a n) -> p a n", a=CCH)
                    wqv = wq8.rearrange("p (a i) -> p a i", a=CCH)
                    for k in range(2):
                        nc.tensor.matmul(
                            acc[:], wqv[:, 2 * k:2 * k + 2, i * 128:(i + 1) * 128],
                            x8v[:, 2 * k:2 * k + 2, :],
                            start=(k == 0), stop=(k == 1), perf_mode=DR)
                    qf = qp.tile([128, 512], FP8, tag="qf", name="qf", bufs=2)
                    nc.vector.tensor_copy(qf[:], acc[:])
                    if _DEBUG and i == 0 and qts is qts0_dbg[0]:
                        nc.gpsimd.dma_start(out=dbg["d_qf"][:], in_=qf[:])
                        nc.gpsimd.dma_start(out=dbg["d_wq8"][:], in_=wq8[:])
                    qt = qp.tile([64, 1024], FP8, tag="q", name="qt")
                    nc.sync.dma_start(
                        out=qt.rearrange("p (j n) -> p j n", j=2), in_=qf[:])
                    qts.append(qt)

                # ---------------- kT = Wk^T ctx^T ; v' = (ctx Wv | 1) --------
                # Ordered for pipeline lead-in: all input DMAs first, then
                # kT i-chunk 0 (both m halves) so attention head 0 can start,
                # then v' (needed by the first AV), then kT i-chunks 1-3.
                prolog_cm = tc.tile_pool(name="prolog", bufs=1)
                prolog = prolog_cm.__enter__()
                ktf8 = [prolog.tile([128, 1024], FP8, tag=f"ktf{i}",
                                    name=f"ktf{i}") for i in range(CCH)]
                # wk/ctx first on the two HWDGE queues (kT chunk 0 is the
                # critical path to the first exp); wv/wo/bo on SWDGE.
                wks = prolog.tile([128, XCH * INNER], F32R, tag="wks", name="wks")
                nc.sync.dma_start(
                    out=wks.rearrange("p (a i) -> p a i", a=XCH),
                    in_=wkr[:])
                ctxs = [prolog.tile([128, XCH * 512], F32R, tag=f"ctxs{mh}", name="ctxs")
                        for mh in range(2)]
                for mh in range(2):
                    nc.scalar.dma_start(
                        out=ctxs[mh].rearrange("p (a m) -> p a m", a=XCH),
                        in_=ctxr[:, :, mh * 512:(mh + 1) * 512])
                wqs = prolog.tile([128, CCH * INNER], F32R, tag="wqs", name="wqs")
                nc.sync.dma_start(
                    out=wqs.rearrange("p (a i) -> p a i", a=CCH), in_=wqr[:])
                wvs = prolog.tile([128, XCH * INNER], F32R, tag="wvs", name="wvs")
                nc.gpsimd.dma_start(
                    out=wvs.rearrange("p (a i) -> p a i", a=XCH), in_=wvr[:])
                wk8 = prolog.tile([128, XCH * INNER], FP8, tag="wk8", name="wk8")
                nc.vector.tensor_copy(wk8[:], wks[:])
                ctx8 = [prolog.tile([128, XCH * 512], FP8, tag=f"ctx8{mh}", name="ctx8")
                        for mh in range(2)]
                nc.vector.tensor_copy(ctx8[0][:], ctxs[0][:])
                wq8 = pers.tile([128, CCH * INNER], FP8, tag="wq8", name="wq8")
                nc.vector.tensor_copy(wq8[:], wqs[:])
                xts0, x8_0 = qproj_load(0)
                nc.vector.tensor_copy(ctx8[1][:], ctxs[1][:])
                wv8 = prolog.tile([128, XCH * INNER], FP8, tag="wv8", name="wv8")
                nc.vector.tensor_copy(wv8[:], wvs[:])
                for c in range(CCH):
                    wos = outp.tile([128, C], F32, tag="ott", name="wos")
                    nc.gpsimd.dma_start(out=wos, in_=wor[c])
                    nc.vector.tensor_copy(wo_bf[c], wos)
                nc.gpsimd.dma_start(out=bo_sb, in_=bo_d.rearrange("(a p) -> p a", p=128))

                def kproj_i(i):
                    wkv = wk8.rearrange("p (a i) -> p a i", a=XCH)
                    for mh in range(2):
                        acc = psA.tile([128, 512], F32, tag="acc", name="kacc")
                        cxv = ctx8[mh].rearrange("p (a m) -> p a m", a=XCH)
                        for k in range(XCH // 2):
                            nc.tensor.matmul(
                                acc[:],
                                wkv[:, 2 * k:2 * k + 2, i * 128:(i + 1) * 128],
                                cxv[:, 2 * k:2 * k + 2, :],
                                start=(k == 0), stop=(k == XCH // 2 - 1),
                                perf_mode=DR)
                        nc.vector.tensor_copy(
                            ktf8[i][:, mh * 512:(mh + 1) * 512], acc[:])
                    nc.sync.dma_start(
                        out=kt8[i].rearrange("p (j n) -> p j n", j=2),
                        in_=ktf8[i][:])

                def vproj_m(m):
                    mh, ml = m // 4, m % 4
                    acc = psA.tile([128, 512], F32, tag="acc", name="vacc")
                    cxv = ctx8[mh].rearrange("p (a m) -> p a m", a=XCH)
                    wvv = wv8.rearrange("p (a i) -> p a i", a=XCH)
                    for k in range(XCH // 2):
                        nc.tensor.matmul(
                            acc[:],
                            cxv[:, 2 * k:2 * k + 2, ml * 128:(ml + 1) * 128],
                            wvv[:, 2 * k:2 * k + 2, :],
                            start=(k == 0), stop=(k == XCH // 2 - 1),
                            perf_mode=DR)
                    vdst = v2_sb[m // 2].rearrange(
                        "p (i h j) -> p i h j", i=2, j=66)[:, m % 2]
                    nc.vector.tensor_copy(
                        vdst[:, :, 0:DH],
                        acc.rearrange("p (h j) -> p h j", j=DH))
                    nc.vector.tensor_copy(
                        vdst[:, :, DH:DH + 1],
                        ones_sb.rearrange("p (h j) -> p h j", j=1))

                kproj_i(0)
                qts0 = []
                qts0_dbg[0] = qts0
                for i in range(CCH):
                    qproj_i(i, x8_0, qts0)
                # vproj + kT i-chunks 1-3 are interleaved into n-chunk 0's
                # head loop (see below) so the exp stream starts immediately
                # after kT[0]/q[0] instead of behind 84 projection matmuls.

                # ---------------- pipeline stages ----------------
                def attn_qk(nci, h, qts):
                    """S^T = kT_h^T q_h (fp8 DoubleRow, K=64 as 32x2) ->
                    exp -> fp8 P^T tiles (4 groups)."""
                    hb = (h % 2) * 32
                    qh = qts[h // 2].rearrange(
                        "p (j n) -> p j n", j=2)[hb:hb + 32]
                    kth = kt8[h // 2].rearrange("p (j n) -> p j n", j=2)
                    pts = []
                    for g in range(NG):
                        st = psS.tile([128, 1024], F32, tag="s", name="st")
                        for j in range(2):
                            m = 2 * g + j
                            nc.tensor.matmul(
                                st[:, j * 512:(j + 1) * 512],
                                kth[hb:hb + 32, :, m * 128:(m + 1) * 128],
                                qh, start=True, stop=True, perf_mode=DR)
                        pt = pp.tile([128, 1024], FP8, tag="p", name="pt")
                        nc.scalar.activation(pt[:], st[:], AF.Exp, scale=SCALE,
                                             bias=ebias[:])
                        if _DEBUG and nci == 0 and h == 0 and g == 0:
                            nc.gpsimd.dma_start(out=dbg["d_pt"][:], in_=pt[:])
                        pts.append(pt)
                    return pts

                def attn_av(h, pts, o_nc, sall):
                    """O' = v'^T P^T accumulated over m; evacuate + denoms."""
                    acc = psA.tile([128, 512], F32, tag="acc", name="avacc")
                    for g in range(NG):
                        v2h = v2_sb[g].rearrange(
                            "p (i h j) -> p i h j", i=2, j=66)[:, :, h, 0:DH + 1]
                        nc.tensor.matmul(
                            acc[0:DH + 1, :], v2h,
                            pts[g].rearrange("p (i n) -> p i n", i=2),
                            start=(g == 0), stop=(g == NG - 1), perf_mode=DR)
                    k = h // 2
                    if h % 2 == 0:
                        nc.vector.tensor_copy(o_nc[k][0:64, :], acc[0:64, :])
                    else:
                        ot = ostp.tile([64, 512], BF16, tag="oev", name="oev")
                        nc.vector.tensor_copy(ot[:], acc[0:64, :])
                        nc.sync.dma_start(out=o_nc[k][64:128, :], in_=ot[:])
                    nc.vector.tensor_copy(
                        sall[64:65, h * 512:(h + 1) * 512], acc[64:65, :])

                def oproj_c(nci, c, o_nc, xts):
                    """one c-chunk of out[:, nsl] = Wo^T O_norm + bo + x."""
                    nsl = slice(nci * 512, (nci + 1) * 512)
                    acc = psA.tile([128, 512], F32, tag="acc", name="oacc")
                    for k in range(CCH):
                        nc.tensor.matmul(
                            acc[:], wo_bf[k][:, c * 128:(c + 1) * 128],
                            o_nc[k][:], start=(k == 0), stop=(k == CCH - 1))
                    ott = outp.tile([128, 512], F32, tag="ott", name="ott")
                    nc.vector.scalar_tensor_tensor(
                        out=ott[:], in0=acc[:], scalar=bo_sb[:, c:c + 1],
                        in1=xts[:, c * 512:(c + 1) * 512].bitcast(F32),
                        op0=ALU.add, op1=ALU.add)
                    nc.sync.dma_start(
                        out=out_d[c * 128:(c + 1) * 128, nsl], in_=ott[:])

                # epilogue of chunk nci, stage s (spread over next chunk's
                # head loop so DVE never queue-head-blocks on a DMA)
                def epilogue(nci, s, state):
                    if s == 0:
                        sums_nc = sump.tile([8, 512], F32, tag="sums",
                                            name="sums")
                        nc.gpsimd.dma_start(out=sums_nc[:],
                                            in_=state["sall"][64:65, :])
                        state["sums"] = sums_nc
                    elif s == 1:
                        rec8 = sump.tile([8, 512], F32, tag="rec", name="rec")
                        recb8 = sump.tile([8, 512], BF16, tag="recb",
                                          name="recb")
                        nc.vector.reciprocal_approx_fast(out=rec8[:],
                                                         in_=state["sums"][:])
                        nc.vector.tensor_copy(recb8[:], rec8[:])
                        state["recb"] = recb8
                    elif s == 2:
                        r0 = nci * 8
                        nc.gpsimd.dma_start(out=rscr_d[r0:r0 + 8, :],
                                            in_=state["recb"][:])
                        rr = rrp.tile([128, 2048], BF16, tag="rrep", name="rr")
                        for par in range(2):
                            src = bass.AP(
                                tensor=rscr_d[:].tensor,
                                offset=rscr_d[r0 + par:r0 + par + 1, :].offset,
                                ap=[[0, 64], [1024, 4], [1, 512]])
                            nc.gpsimd.dma_start(
                                out=rr[par * 64:(par + 1) * 64, :], in_=src)
                        state["rr"] = rr
                    elif s == 3:
                        for k in range(CCH):
                            nc.vector.tensor_mul(
                                state["o_nc"][k][:], state["o_nc"][k][:],
                                state["rr"][:, k * 512:(k + 1) * 512])
                        if _DEBUG and nci == 0:
                            nc.gpsimd.dma_start(out=dbg["d_onc"][:],
                                                in_=state["o_nc"][0][:])
                            nc.gpsimd.dma_start(out=dbg["d_rr"][:],
                                                in_=state["rr"][:])

                # ---------------- main software-pipelined loop ----------------
                qts = qts0
                xts = xts0
                prev = None     # epilogue/oproj state of chunk nci-1
                for nci in range(NCI):
                    if nci < NCI - 1:
                        xts_next, x8_next = qproj_load(nci + 1)
                    qts_next = []
                    o_nc = [oncp.tile([128, 512], BF16, tag=f"onc{k}",
                                      name=f"onc{k}") for k in range(CCH)]
                    sall = sump.tile([65, 4096], F32, tag="sall", name="sall",
                     bufs=1)
                    pts_prev = None
                    for h in range(HEADS):
                        pts = attn_qk(nci, h, qts)
                        if nci == 0:
                            if h == 0:
                                for m in range(MCI):
                                    vproj_m(m)
                            elif h <= 3:
                                kproj_i(h)
                        if pts_prev is not None:
                            attn_av(h - 1, pts_prev, o_nc, sall)
                        pts_prev = pts
                        if prev is not None and 1 <= h <= 3:
                            epilogue(nci - 1, h, prev)
                        if nci < NCI - 1 and h < CCH:
                            qproj_i(h, x8_next, qts_next)
                        if prev is not None and h >= 4:
                            oproj_c(nci - 1, h - 4, prev["o_nc"], prev["xts"])
                    attn_av(HEADS - 1, pts_prev, o_nc, sall)
                    if _DEBUG and nci == 0:
                        nc.gpsimd.dma_start(out=dbg["d_qt"][:], in_=qts[0][:])
                        nc.gpsimd.dma_start(out=dbg["d_kt8"][:], in_=kt8[0][:])
                        nc.gpsimd.dma_start(out=dbg["d_sall"][:], in_=sall[64:65, :])
                        nc.gpsimd.dma_start(
                            out=dbg["d_v2"].rearrange(
                                "p (i h j) -> p i h j", i=2, j=65),
                            in_=v2_sb[0].rearrange(
                                "p (i h j) -> p i h j", i=2, j=66)[:, :, :, 0:65])
                        nc.gpsimd.dma_start(out=dbg["d_x8"][:], in_=x8_0[:])
                    prev = {"sall": sall, "o_nc": o_nc, "xts": xts}
                    epilogue(nci, 0, prev)
                    if nci == NCI - 1:
                        for s in (1, 2, 3):
                            epilogue(nci, s, prev)
                        for c in range(CCH):
                            oproj_c(nci, c, o_nc, xts)
                    qts = qts_next
                    xts = xts_next if nci < NCI - 1 else xts
                    if nci == 0:
                        prolog_cm.__exit__(None, None, None)

    nc.compile()
    return nc


def kernel(x, context, Wq, Wk, Wv, Wo, bo):
    global _PROG
    if _PROG is None:
        _PROG = _build()
    nc = _PROG

    x = np.asarray(x, np.float32).reshape(B, C, N)
    ctxT = np.ascontiguousarray(
        np.asarray(context, np.float32).transpose(0, 2, 1))
    wq = np.ascontiguousarray(np.asarray(Wq, np.float32))
    wk = np.ascontiguousarray(np.asarray(Wk, np.float32))
    wv = np.ascontiguousarray(np.asarray(Wv, np.float32))
    wo = np.ascontiguousarray(np.asarray(Wo, np.float32))
    bov = np.ascontiguousarray(np.asarray(bo, np.float32))

    in_maps = [
        {"x": np.ascontiguousarray(x[b]), "ctxT": ctxT[b],
         "wq": wq, "wk": wk, "wv": wv, "wo": wo, "bo": bov}
        for b in range(B)
    ]
    res = run_bass_kernel_spmd(nc, in_maps, core_ids=list(range(8)))
    out = np.stack([res.results[b]["out"] for b in range(B)], axis=0)
    return out.reshape(B, C, HH, WW).astype(np.float32)
